# revision 18
# baseline (speedup 1.0000x reference)
"""Trainium2 Bass kernel for a 12-layer BERT encoder + ragged segment-mean pooling.

Sharding: data-parallel over batch - 8 sequences, one per NeuronCore. Each core
runs the full encoder on its sequence with replicated weights streamed from HBM
in bf16, then pools subwords -> 128 tokens with a host-precomputed pooling
matrix on the tensor engine.

Key layout/speed choices per core:
  - residual stream h/t/h2 kept token-major [128part, 2chunk, 768] in bf16
    (DVE gets 2-4x throughput on 2-byte dtypes; LN stats via bn_stats/bn_aggr
    in fp32; 1/sqrt(var) computed as exp(-0.5*ln(var+eps)) so the activation
    table never leaves the exp/ln set except for GELU)
  - activation transposes (hT/h2T) done by the DMA xbar engine (bf16), not PE
  - attention in "scoresT" orientation (keys on partitions). exp() output and
    the v matrix are fp8e4m3; each head's context AND softmax denominator come
    from a single DoubleRow fp8 matmul (v gets a ones column appended), the
    denominator reciprocal is broadcast across partitions with a tiny
    2-partition matmul per head pair
  - free dims covering subword positions are trimmed to SQ=192 (actual max
    valid length 190 < 192; falls back to 256 if inputs exceed it)
  - the 1/sqrt(64) query scale is folded into the q weights on the host
"""

import numpy as np
import ml_dtypes

H = 768
L = 12
NH = 12
HD = 64
FF = 3072
VOCAB = 30522
BZ = 8
S = 256
T = 128
EPS = 1e-12
P = 128
NCHUNK = S // P  # 2 token chunks of 128
KT = H // P      # 6 contraction tiles over hidden dim
FKT = FF // P    # 24 contraction tiles over ffn dim

BF16 = np.float16
F8 = ml_dtypes.float8_e4m3

import os as _os

# dev knob: build with fewer encoder layers (timing experiments only)
L_BUILD = int(_os.environ.get("KERNEL_L_BUILD", str(L)))
# dev knob: wrap the encoder-layer stack in a hardware loop that runs it R
# times, to measure per-iteration HW time above the axon dispatch overhead
TIME_LOOP = int(_os.environ.get("KERNEL_TIME_LOOP", "0"))

_CACHE = {}
_CAST_CACHE = {}


def _build_program(flags, time_loop=None):
    time_loop = TIME_LOOP if time_loop is None else time_loop
    import concourse.bacc as bacc
    import concourse.mybir as mybir
    import concourse.tile as tile

    dt = mybir.dt
    AF = mybir.ActivationFunctionType
    OP = mybir.AluOpType
    DR = mybir.MatmulPerfMode.DoubleRow

    SQ = flags["sq"]  # trimmed subword free-dim (queries/ffn tokens)

    nc = bacc.Bacc(
        "TRN2",
        target_bir_lowering=False,
        debug=False,
        enable_asserts=False,
        num_devices=8,
    )

    # ---- DRAM I/O ----
    ids_d = nc.dram_tensor("ids32", (NCHUNK, P), dt.int32, kind="ExternalInput").ap()
    seg_d = nc.dram_tensor("seg32", (NCHUNK, P), dt.int32, kind="ExternalInput").ap()
    am_d = nc.dram_tensor("am", (P, NCHUNK), dt.float32, kind="ExternalInput").ap()
    atp_d = nc.dram_tensor("atp", (P, NCHUNK, T), dt.float16, kind="ExternalInput").ap()
    wemb_d = nc.dram_tensor("word_emb", (VOCAB, H), dt.float16, kind="ExternalInput").ap()
    pemb_d = nc.dram_tensor("pos_emb", (S, H), dt.float16, kind="ExternalInput").ap()
    temb_d = nc.dram_tensor("type_emb", (2, H), dt.float16, kind="ExternalInput").ap()
    qkvw_d = nc.dram_tensor("qkv_w", (L, H, 3 * H), dt.float16, kind="ExternalInput").ap()
    ow_d = nc.dram_tensor("attn_out_w", (L, H, H), dt.float16, kind="ExternalInput").ap()
    f1w_d = nc.dram_tensor("ff1_w", (L, H, FF), dt.float16, kind="ExternalInput").ap()
    f2w_d = nc.dram_tensor("ff2_w", (L, FF, H), dt.float16, kind="ExternalInput").ap()

    # optional (only present when the corresponding values are nontrivial)
    gb_d = {}
    if flags["emb_gb"]:
        gb_d["emb_g"] = nc.dram_tensor("emb_g", (H,), dt.float32, kind="ExternalInput").ap()
        gb_d["emb_b"] = nc.dram_tensor("emb_b", (H,), dt.float32, kind="ExternalInput").ap()
    if flags["ln1_gb"]:
        gb_d["ln1_g"] = nc.dram_tensor("ln1_g", (L, H), dt.float32, kind="ExternalInput").ap()
        gb_d["ln1_b"] = nc.dram_tensor("ln1_b", (L, H), dt.float32, kind="ExternalInput").ap()
    if flags["ln2_gb"]:
        gb_d["ln2_g"] = nc.dram_tensor("ln2_g", (L, H), dt.float32, kind="ExternalInput").ap()
        gb_d["ln2_b"] = nc.dram_tensor("ln2_b", (L, H), dt.float32, kind="ExternalInput").ap()
    if flags["qkv_b"]:
        gb_d["qkv_b"] = nc.dram_tensor("qkv_b", (L, 3 * H), dt.float32, kind="ExternalInput").ap()
    if flags["ob"]:
        gb_d["ob"] = nc.dram_tensor("ob", (L, H), dt.float32, kind="ExternalInput").ap()
    if flags["f1b"]:
        gb_d["f1b"] = nc.dram_tensor("f1b", (L, FF), dt.float32, kind="ExternalInput").ap()
    if flags["f2b"]:
        gb_d["f2b"] = nc.dram_tensor("f2b", (L, H), dt.float32, kind="ExternalInput").ap()

    out_d = nc.dram_tensor("outp", (T, H), dt.float32, kind="ExternalOutput").ap()

    with tile.TileContext(nc) as tc:
        from contextlib import ExitStack

        ctx = ExitStack()
        with ctx:
            persist = ctx.enter_context(tc.tile_pool(name="persist", bufs=1))
            wpool = ctx.enter_context(tc.tile_pool(name="wpool", bufs=16))
            work = ctx.enter_context(tc.tile_pool(name="work", bufs=3))
            ps = ctx.enter_context(tc.tile_pool(name="ps", bufs=3, space="PSUM"))
            psc = ctx.enter_context(tc.tile_pool(name="psc", bufs=2, space="PSUM"))
            pcx = ctx.enter_context(tc.tile_pool(name="pcx", bufs=3, space="PSUM"))

            # ---- persistent tiles ----
            am_sb = persist.tile([P, NCHUNK], dt.float32, tag="am_sb")
            nc.sync.dma_start(am_sb, am_d)
            eps_sb = persist.tile([P, 1], dt.float32, tag="eps_sb")
            nc.vector.memset(eps_sb, EPS)
            c15 = persist.tile([P, 1], dt.float32, tag="c15")
            nc.vector.memset(c15, 1.5)
            cmagic = persist.tile([P, 1], dt.int32, tag="cmagic")
            nc.vector.memset(cmagic, 0x5F3759DF)
            rvps = []
            for i in range(2):
                rvp_t = persist.tile([33, SQ], dt.float16, tag=f"rvp{i}")
                nc.vector.memset(rvp_t, 0.0)
                rvps.append(rvp_t)
            atp_sb = persist.tile([P, NCHUNK, T], dt.float16, tag="atp_sb")
            nc.sync.dma_start(atp_sb, atp_d)
            from concourse.masks import make_identity

            ident = persist.tile([P, P], dt.float16, tag="ident")
            make_identity(nc, ident)
            # head-pair broadcast matrix: row0 -> partitions 0-63, row1 -> 64-127
            blk = persist.tile([33, P], dt.float16, tag="blk")
            nc.vector.memset(blk, 0.0)
            nc.vector.memset(blk[0:1, 0:HD], 1.0)
            nc.vector.memset(blk[32:33, HD:P], 1.0)

            h = persist.tile([P, NCHUNK, H], dt.float16, tag="h")
            t = persist.tile([P, NCHUNK, H], dt.float16, tag="t")
            h2 = persist.tile([P, NCHUNK, H], dt.float16, tag="h2")
            hT = persist.tile([P, KT, S], dt.float16, tag="hT")
            h2T = persist.tile([P, KT, S], dt.float16, tag="h2T")
            qT = persist.tile([P, KT, SQ], dt.float16, tag="qT")
            kTt = persist.tile([P, KT, SQ], dt.float16, tag="kTt")
            # v with a ones column per head: fused context+denominator matmul
            vtm = persist.tile([P, NCHUNK, NH, HD + 32], dt.float8e4, tag="vtm")
            nc.vector.memset(vtm, 0.0)
            for hd in range(NH):
                nc.vector.memset(vtm[:, :, hd, HD:HD + 1], 1.0)
            ctxT = persist.tile([P, KT, S], dt.float16, tag="ctxT")
            nc.vector.memset(ctxT, 0.0)
            fT = persist.tile([P, FKT, SQ], dt.float16, tag="fT")
            out_sb = persist.tile([P, H], dt.float32, tag="out_sb")

            # broadcast (across partitions) gain/bias tiles, if nontrivial
            def bcast_load(name, src_ap, width):
                til = persist.tile([P, width], dt.float32, tag=name, name=name)
                import concourse.bass as bass

                bap = bass.AP(
                    tensor=src_ap.tensor,
                    offset=src_ap.offset,
                    ap=[[0, P]] + list(src_ap.ap),
                )
                nc.gpsimd.dma_start(out=til, in_=bap)
                return til

            gb_sb = {}
            if flags["emb_gb"]:
                gb_sb["emb_g"] = bcast_load("emb_g_sb", gb_d["emb_g"], H)
                gb_sb["emb_b"] = bcast_load("emb_b_sb", gb_d["emb_b"], H)
            if flags["ln1_gb"]:
                gb_sb["ln1_g"] = bcast_load("ln1_g_sb", gb_d["ln1_g"].rearrange("l h -> (l h)"), L * H)
                gb_sb["ln1_b"] = bcast_load("ln1_b_sb", gb_d["ln1_b"].rearrange("l h -> (l h)"), L * H)
            if flags["ln2_gb"]:
                gb_sb["ln2_g"] = bcast_load("ln2_g_sb", gb_d["ln2_g"].rearrange("l h -> (l h)"), L * H)
                gb_sb["ln2_b"] = bcast_load("ln2_b_sb", gb_d["ln2_b"].rearrange("l h -> (l h)"), L * H)
            if flags["ob"]:
                gb_sb["ob"] = bcast_load("ob_sb", gb_d["ob"].rearrange("l h -> (l h)"), L * H)
            if flags["f2b"]:
                gb_sb["f2b"] = bcast_load("f2b_sb", gb_d["f2b"].rearrange("l h -> (l h)"), L * H)
            if flags["qkv_b"]:
                qkvb_sb = persist.tile([P, L, 3 * H // P], dt.float32, tag="qkvb_sb")
                nc.sync.dma_start(qkvb_sb, gb_d["qkv_b"].rearrange("l (o p) -> p l o", p=P))
                gb_sb["vb"] = bcast_load("vb_sb", gb_d["qkv_b"].rearrange("l h -> (l h)"), L * 3 * H)
            if flags["f1b"]:
                f1b_sb = persist.tile([P, L, FF // P], dt.float32, tag="f1b_sb")
                nc.sync.dma_start(f1b_sb, gb_d["f1b"].rearrange("l (o p) -> p l o", p=P))

            # ---- layernorm helper: src bf16 [P, H] slice -> dst bf16 [P, H] ----
            # rstd = exp(-0.5 * ln(var + eps)) keeps the act table on exp/ln.
            def layer_norm(src, dst, g_ap, b_ap, stats=None):
                if stats is None:
                    stats = work.tile([P, 2, 6], dt.float32, tag="st", name="stats")
                    for sg in range(2):
                        nc.vector.bn_stats(stats[:, sg, :], src[:, sg * 384:(sg + 1) * 384])
                mv = work.tile([P, 2], dt.float32, tag="mv", name="mv")
                nc.vector.bn_aggr(mv, stats)
                # rstd = 1/sqrt(var) via bit-trick seed + 2 Newton steps (DVE only)
                rstd = work.tile([P, 4], dt.float32, tag="rs", name="rstd")
                iv = rstd[:, 3:4].bitcast(dt.int32)
                nc.vector.tensor_scalar(
                    out=iv, in0=mv[:, 1:2].bitcast(dt.int32),
                    scalar1=1, scalar2=None, op0=OP.arith_shift_right,
                )
                nc.vector.scalar_tensor_tensor(
                    rstd[:, 0:1].bitcast(dt.int32), iv, -1, cmagic,
                    op0=OP.mult, op1=OP.add,
                )
                nc.vector.tensor_scalar(
                    out=rstd[:, 1:2], in0=mv[:, 1:2],
                    scalar1=-0.5, scalar2=None, op0=OP.mult,
                )
                for _it in range(2):
                    nc.vector.tensor_tensor(rstd[:, 2:3], rstd[:, 0:1], rstd[:, 0:1], op=OP.mult)
                    nc.vector.scalar_tensor_tensor(
                        rstd[:, 2:3], rstd[:, 2:3], rstd[:, 1:2], c15,
                        op0=OP.mult, op1=OP.add,
                    )
                    nc.vector.tensor_tensor(rstd[:, 0:1], rstd[:, 0:1], rstd[:, 2:3], op=OP.mult)
                nc.vector.tensor_scalar(
                    out=dst,
                    in0=src,
                    scalar1=mv[:, 0:1],
                    scalar2=rstd[:, 0:1],
                    op0=OP.subtract,
                    op1=OP.mult,
                )
                if g_ap is not None:
                    nc.vector.tensor_tensor(dst, dst, g_ap, op=OP.mult)
                if b_ap is not None:
                    nc.vector.tensor_tensor(dst, dst, b_ap, op=OP.add)

            # transpose [P, H] chunk of src into dstT[:, o, c*128:(c+1)*128]
            # (PE transpose in bf16; psum->sbuf copy on the Pool engine)
            def dma_transpose(src_c, dstT, c):
                for o in range(KT):
                    tp = pcx.tile([P, P], dt.float16, tag="cx", name="tp")
                    nc.tensor.transpose(tp, src_c[:, o * P:(o + 1) * P], ident)
                    if o % 2 == 0:
                        nc.scalar.copy(dstT[:, o, c * P:(c + 1) * P], tp)
                    else:
                        nc.vector.tensor_copy(dstT[:, o, c * P:(c + 1) * P], tp)

            # ---- embedding: gather + add + LN -> h ----
            for c in range(NCHUNK):
                idt = work.tile([P, 1], dt.int32, tag="idt", name="idt")
                nc.sync.dma_start(idt, ids_d[c, :, None])
                gat = work.tile([P, H], dt.float16, tag="gat", name="gat")
                import concourse.bass as bass

                nc.gpsimd.indirect_dma_start(
                    out=gat,
                    out_offset=None,
                    in_=wemb_d[:],
                    in_offset=bass.IndirectOffsetOnAxis(ap=idt[:, :1], axis=0),
                )
                sgt = work.tile([P, 1], dt.int32, tag="idt", name="sgt")
                nc.sync.dma_start(sgt, seg_d[c, :, None])
                gat2 = work.tile([P, H], dt.float16, tag="gat", name="gat2")
                nc.gpsimd.indirect_dma_start(
                    out=gat2,
                    out_offset=None,
                    in_=temb_d[:],
                    in_offset=bass.IndirectOffsetOnAxis(ap=sgt[:, :1], axis=0),
                )
                pos = work.tile([P, H], dt.float16, tag="gat", name="pos")
                nc.sync.dma_start(pos, pemb_d[c * P:(c + 1) * P, :])
                nc.vector.tensor_tensor(t[:, c, :], gat, gat2, op=OP.add)
                nc.vector.tensor_tensor(t[:, c, :], t[:, c, :], pos, op=OP.add)
                layer_norm(
                    t[:, c, :],
                    h[:, c, :],
                    gb_sb.get("emb_g"),
                    gb_sb.get("emb_b"),
                )
                dma_transpose(h[:, c, :], hT, c)

            # ---- encoder layers ----
            def _layer_stack():
                for l in range(L_BUILD):
                    qkvw_l = qkvw_d[l].rearrange("(o p) n -> p o n", p=P)
                    ow_l = ow_d[l].rearrange("(o p) n -> p o n", p=P)
                    f1w_l = f1w_d[l].rearrange("(o p) n -> p o n", p=P)
                    f2w_l = f2w_d[l].rearrange("(o p) n -> p o n", p=P)

                    # --- B: qT (scale pre-folded), kTt, v ---
                    for ch in range(3):
                        wq = wpool.tile([P, KT, H], dt.float16, tag="w", name="wq")
                        nc.sync.dma_start(wq, qkvw_l[:, :, ch * H:(ch + 1) * H])
                        if ch < 2:
                            dst, n = (qT, SQ) if ch == 0 else (kTt, SQ)
                            for oc in range(KT):
                                pt = ps.tile([P, S], dt.float32, tag="acc", name="pt")
                                for kt in range(KT):
                                    nc.tensor.matmul(
                                        pt[:, :n],
                                        lhsT=wq[:, kt, oc * P:(oc + 1) * P],
                                        rhs=hT[:, kt, :n],
                                        start=(kt == 0),
                                        stop=(kt == KT - 1),
                                    )
                                if flags["qkv_b"]:
                                    nc.scalar.activation(
                                        dst[:, oc, :], pt[:, :n], AF.Identity,
                                        bias=qkvb_sb[:, l, ch * KT + oc:ch * KT + oc + 1],
                                        scale=1.0,
                                    )
                                else:
                                    nc.vector.tensor_copy(dst[:, oc, :], pt[:, :n])
                        else:
                            for mc in range(NCHUNK):
                                for n2 in range(2):
                                    pt = ps.tile([P, KT, HD], dt.float32, tag="acc", name="ptv")
                                    for kt in range(KT):
                                        nc.tensor.matmul(
                                            pt,
                                            lhsT=hT[:, kt, mc * P:(mc + 1) * P],
                                            rhs=wq[:, kt, n2 * 384:(n2 + 1) * 384],
                                            start=(kt == 0),
                                            stop=(kt == KT - 1),
                                        )
                                    if flags["qkv_b"]:
                                        vb = gb_sb["vb"]
                                        base = l * 3 * H + 2 * H + n2 * 384
                                        vb_ap = vb[:, base:base + 384]
                                        tmp = work.tile([P, KT, HD], dt.float32, tag="vtmp", name="vtmp")
                                        nc.vector.tensor_tensor(tmp, pt, vb_ap, op=OP.add)
                                        nc.vector.tensor_copy(
                                            vtm[:, mc, n2 * 6:(n2 + 1) * 6, 0:HD], tmp
                                        )
                                    else:
                                        nc.scalar.copy(
                                            vtm[:, mc, n2 * 6:(n2 + 1) * 6, 0:HD], pt
                                        )

                    # --- C: attention, head pairs ---
                    for o in range(KT):
                        rvp = rvps[o % 2]
                        cxs = []
                        for sub in range(2):
                            hd = 2 * o + sub
                            r0 = sub * HD
                            sc = psc.tile([P, NCHUNK, SQ], dt.float32, tag="sc", name="sc")
                            for kc in range(NCHUNK):
                                k1 = min((kc + 1) * P, SQ)
                                nc.tensor.matmul(
                                    sc[: k1 - kc * P, kc, :],
                                    lhsT=kTt[r0:r0 + HD, o, kc * P:k1],
                                    rhs=qT[r0:r0 + HD, o, :],
                                    start=True,
                                    stop=True,
                                )
                            e = work.tile([P, NCHUNK, SQ], dt.float8e4, tag="e", name="e")
                            for kc in range(NCHUNK):
                                nc.scalar.activation(
                                    e[:, kc, :], sc[:, kc, :], AF.Exp,
                                    bias=am_sb[:, kc:kc + 1], scale=1.0,
                                )
                            cx = pcx.tile([HD + 32, SQ], dt.float32, tag="cx", name="cx")
                            nc.tensor.matmul(
                                cx, lhsT=vtm[:, :, hd, :], rhs=e,
                                start=True, stop=True, perf_mode=DR,
                            )
                            with nc.allow_low_precision(reason="bf16 softmax denom"):
                                nc.vector.reciprocal(rvp[sub * 32:sub * 32 + 1, :], cx[HD:HD + 1, :])
                            cxs.append(cx)
                        bc = pcx.tile([P, SQ], dt.float32, tag="cx", name="bc")
                        nc.tensor.matmul(bc, lhsT=blk, rhs=rvp, start=True, stop=True)
                        bc_sb = work.tile([P, SQ], dt.float16, tag="bcs", name="bc_sb")
                        nc.scalar.copy(bc_sb, bc)
                        for sub in range(2):
                            r0 = sub * HD
                            nc.vector.tensor_tensor(
                                ctxT[r0:r0 + HD, o, :SQ],
                                cxs[sub][0:HD, :],
                                bc_sb[r0:r0 + HD, :],
                                op=OP.mult,
                            )

                    # --- D: attn out + residual + LN1 ---
                    wo = wpool.tile([P, KT, H], dt.float16, tag="w", name="wo")
                    nc.sync.dma_start(wo, ow_l)
                    for mc in range(NCHUNK):
                        stats = work.tile([P, 2, 6], dt.float32, tag="st", name="stats")
                        for n2 in range(2):
                            pt = ps.tile([P, 384], dt.float32, tag="acc", name="pta")
                            for kt in range(KT):
                                nc.tensor.matmul(
                                    pt,
                                    lhsT=ctxT[:, kt, mc * P:(mc + 1) * P],
                                    rhs=wo[:, kt, n2 * 384:(n2 + 1) * 384],
                                    start=(kt == 0),
                                    stop=(kt == KT - 1),
                                )
                            sl = slice(n2 * 384, (n2 + 1) * 384)
                            nc.vector.tensor_tensor(t[:, mc, sl], h[:, mc, sl], pt, op=OP.add)
                            if flags["ob"]:
                                ob = gb_sb["ob"]
                                nc.vector.tensor_tensor(
                                    t[:, mc, sl], t[:, mc, sl],
                                    ob[:, l * H + n2 * 384:l * H + (n2 + 1) * 384], op=OP.add,
                                )
                            nc.vector.bn_stats(stats[:, n2, :], t[:, mc, sl])
                        g_ap = gb_sb["ln1_g"][:, l * H:(l + 1) * H] if flags["ln1_gb"] else None
                        b_ap = gb_sb["ln1_b"][:, l * H:(l + 1) * H] if flags["ln1_gb"] else None
                        layer_norm(t[:, mc, :], h2[:, mc, :], g_ap, b_ap, stats=stats)
                        dma_transpose(h2[:, mc, :], h2T, mc)

                    # --- F: FF1 + GELU ---
                    for ch in range(4):
                        w1 = wpool.tile([P, KT, H], dt.float16, tag="w", name="w1")
                        nc.sync.dma_start(w1, f1w_l[:, :, ch * H:(ch + 1) * H])
                        for j in range(3):
                            pt = ps.tile([P, 2, SQ], dt.float32, tag="acc", name="ptf")
                            for f2 in range(2):
                                f_loc = j * 2 + f2
                                for kt in range(KT):
                                    nc.tensor.matmul(
                                        pt[:, f2, :],
                                        lhsT=w1[:, kt, f_loc * P:(f_loc + 1) * P],
                                        rhs=h2T[:, kt, :SQ],
                                        start=(kt == 0),
                                        stop=(kt == KT - 1),
                                    )
                            oc0 = ch * KT + j * 2
                            if flags["f1b"]:
                                for f2 in range(2):
                                    nc.scalar.activation(
                                        fT[:, oc0 + f2, :], pt[:, f2, :], AF.Gelu,
                                        bias=f1b_sb[:, l, oc0 + f2:oc0 + f2 + 1], scale=1.0,
                                    )
                            else:
                                nc.scalar.activation(fT[:, oc0:oc0 + 2, :], pt, AF.Gelu)

                    # --- G: FF2 + residual + LN2 (+ next-layer hT transpose) ---
                    w2c = []
                    for wc in range(4):
                        w2 = wpool.tile([P, KT, H], dt.float16, tag="w", name="w2")
                        nc.sync.dma_start(w2, f2w_l[:, wc * KT:(wc + 1) * KT, :])
                        w2c.append(w2)
                    u2T = work.tile([P, KT, SQ], dt.float16, tag="u2T", name="u2T")
                    for oc in range(KT):
                        pt = ps.tile([P, SQ], dt.float32, tag="acc", name="pt2")
                        for kt in range(FKT):
                            nc.tensor.matmul(
                                pt,
                                lhsT=w2c[kt // KT][:, kt % KT, oc * P:(oc + 1) * P],
                                rhs=fT[:, kt, :SQ],
                                start=(kt == 0),
                                stop=(kt == FKT - 1),
                            )
                        nc.vector.tensor_copy(u2T[:, oc, :], pt)
                    for mc in range(NCHUNK):
                        m0 = mc * P
                        m1 = min((mc + 1) * P, SQ)
                        for o in range(KT):
                            tp = pcx.tile([P, P], dt.float16, tag="cx", name="tpu")
                            nc.tensor.transpose(
                                tp[: m1 - m0, :], u2T[:, o, m0:m1], ident
                            )
                            nc.vector.tensor_tensor(
                                t[:, mc, o * P:(o + 1) * P],
                                h2[:, mc, o * P:(o + 1) * P],
                                tp[:, :],
                                op=OP.add,
                            )
                        if flags["f2b"]:
                            f2b = gb_sb["f2b"]
                            nc.vector.tensor_tensor(
                                t[:, mc, :], t[:, mc, :],
                                f2b[:, l * H:(l + 1) * H], op=OP.add,
                            )
                        g_ap = gb_sb["ln2_g"][:, l * H:(l + 1) * H] if flags["ln2_gb"] else None
                        b_ap = gb_sb["ln2_b"][:, l * H:(l + 1) * H] if flags["ln2_gb"] else None
                        layer_norm(t[:, mc, :], h[:, mc, :], g_ap, b_ap)
                        dma_transpose(h[:, mc, :], hT, mc)

            if time_loop > 0:
                with tc.For_i(0, time_loop, 1):
                    _layer_stack()
            else:
                _layer_stack()

            # ---- pooling: out = A @ enc  (A is host-precomputed, bf16) ----
            for n2 in range(2):
                pt = ps.tile([P, 384], dt.float32, tag="acc", name="ptp")
                for c in range(NCHUNK):
                    nc.tensor.matmul(
                        pt,
                        lhsT=atp_sb[:, c, :],
                        rhs=h[:, c, n2 * 384:(n2 + 1) * 384],
                        start=(c == 0),
                        stop=(c == NCHUNK - 1),
                    )
                nc.vector.tensor_copy(out_sb[:, n2 * 384:(n2 + 1) * 384], pt)
            nc.sync.dma_start(out_d, out_sb)

    nc.finalize()
    return nc


def _host_prep(inputs):
    """Build per-core in_maps from the full inputs."""
    ids = np.asarray(inputs["bert_ids"])
    segs = np.asarray(inputs["segments"])
    mask = np.asarray(inputs["bert_mask"])
    lens = np.asarray(inputs["bert_lens"])
    f32 = lambda k: np.asarray(inputs[k], dtype=np.float32)

    totals = lens.sum(axis=1)
    sq = 192 if totals.max() <= 192 else 256

    flags = {
        "sq": int(sq),
        "emb_gb": not (
            np.all(np.asarray(inputs["emb_ln_g"]) == 1.0)
            and np.all(np.asarray(inputs["emb_ln_b"]) == 0.0)
        ),
        "ln1_gb": not (
            np.all(np.asarray(inputs["ln1_g"]) == 1.0)
            and np.all(np.asarray(inputs["ln1_b"]) == 0.0)
        ),
        "ln2_gb": not (
            np.all(np.asarray(inputs["ln2_g"]) == 1.0)
            and np.all(np.asarray(inputs["ln2_b"]) == 0.0)
        ),
        "qkv_b": bool(np.any(np.asarray(inputs["qkv_b"]) != 0.0)),
        "ob": bool(np.any(np.asarray(inputs["attn_out_b"]) != 0.0)),
        "f1b": bool(np.any(np.asarray(inputs["ff1_b"]) != 0.0)),
        "f2b": bool(np.any(np.asarray(inputs["ff2_b"]) != 0.0)),
    }

    def _bf16(key, arr):
        ck = (key, id(inputs[key]), arr.shape)
        hit = _CAST_CACHE.get(ck)
        if hit is None:
            hit = arr.astype(BF16)
            _CAST_CACHE[ck] = hit
        return hit

    qkv_w = np.asarray(inputs["qkv_w"], dtype=np.float32)
    ck = ("qkv_w_scaled", id(inputs["qkv_w"]), qkv_w.shape)
    qkv_scaled = _CAST_CACHE.get(ck)
    if qkv_scaled is None:
        qkv_scaled = qkv_w.copy()
        qkv_scaled[:, :, :H] *= 0.125  # fold the 1/sqrt(HD) query scale
        qkv_scaled = qkv_scaled.astype(BF16)
        _CAST_CACHE[ck] = qkv_scaled

    shared = {
        "word_emb": _bf16("word_emb", np.asarray(inputs["word_emb"], np.float32)),
        "pos_emb": _bf16("pos_emb", np.ascontiguousarray(np.asarray(inputs["pos_emb"], np.float32)[:S])),
        "type_emb": _bf16("type_emb", np.asarray(inputs["type_emb"], np.float32)),
        "qkv_w": qkv_scaled,
        "attn_out_w": _bf16("attn_out_w", np.asarray(inputs["attn_out_w"], np.float32)),
        "ff1_w": _bf16("ff1_w", np.asarray(inputs["ff1_w"], np.float32)),
        "ff2_w": _bf16("ff2_w", np.asarray(inputs["ff2_w"], np.float32)),
    }
    if flags["emb_gb"]:
        shared["emb_g"] = f32("emb_ln_g")
        shared["emb_b"] = f32("emb_ln_b")
    if flags["ln1_gb"]:
        shared["ln1_g"] = f32("ln1_g")
        shared["ln1_b"] = f32("ln1_b")
    if flags["ln2_gb"]:
        shared["ln2_g"] = f32("ln2_g")
        shared["ln2_b"] = f32("ln2_b")
    if flags["qkv_b"]:
        qb = f32("qkv_b").copy()
        qb[:, :H] *= 0.125
        shared["qkv_b"] = qb
    if flags["ob"]:
        shared["ob"] = f32("attn_out_b")
    if flags["f1b"]:
        shared["f1b"] = f32("ff1_b")
    if flags["f2b"]:
        shared["f2b"] = f32("ff2_b")

    in_maps = []
    for c in range(BZ):
        m = mask[c].astype(np.int64)
        ln = lens[c].astype(np.int64)
        total = int(ln.sum())
        bounds = np.cumsum(ln)
        tok = np.searchsorted(bounds, np.arange(S), side="right")
        tok = np.minimum(tok, T - 1)
        A = np.zeros((T, S), dtype=np.float32)
        for s_i in range(min(total, S)):
            t_i = tok[s_i]
            A[t_i, s_i] = 1.0 / max(int(ln[t_i]), 1)
        atp = np.ascontiguousarray(A.T.reshape(NCHUNK, P, T).transpose(1, 0, 2)).astype(BF16)
        am = ((1.0 - m.astype(np.float32)) * -10000.0).reshape(NCHUNK, P).T
        im = {
            "ids32": ids[c].reshape(NCHUNK, P).astype(np.int32),
            "seg32": segs[c].reshape(NCHUNK, P).astype(np.int32),
            "am": np.ascontiguousarray(am),
            "atp": atp,
        }
        im.update(shared)
        in_maps.append(im)
    return in_maps, flags


def _run(inputs, trace=False):
    from concourse import bass_utils

    in_maps, flags = _host_prep(inputs)
    key = tuple(sorted(flags.items()))
    if key not in _CACHE:
        _CACHE[key] = _build_program(flags)
    nc = _CACHE[key]
    res = bass_utils.run_bass_kernel_spmd(
        nc, in_maps, core_ids=list(range(BZ)), trace=trace
    )
    out = np.stack([np.asarray(res.results[c]["outp"]) for c in range(BZ)], axis=0)
    return out.astype(np.float32), res


def kernel(**inputs):
    import time as _time

    last_err = None
    for attempt in range(3):
        try:
            out, _ = _run(inputs, trace=False)
            return out
        except Exception as e:  # transient device/relay failures
            last_err = e
            _time.sleep(5.0 * (attempt + 1))
    raise last_err


def bench(inputs, iters=10, time_loop=None):
    """Time kernel execution with device-resident inputs (excludes H2D of
    weights). Returns (min_s, all_times, outputs_core0_check)."""
    import time

    import jax
    import jax.numpy as jnp
    from jax.sharding import Mesh, NamedSharding, PartitionSpec

    try:
        from jax.experimental.shard_map import shard_map
    except ImportError:
        from jax.shard_map import shard_map
    import concourse.mybir as mybir
    from concourse import bass2jax

    in_maps, flags = _host_prep(inputs)
    key = (tuple(sorted(flags.items())), time_loop)
    if key not in _CACHE:
        _CACHE[key] = _build_program(flags, time_loop=time_loop)
    nc = _CACHE[key]
    n_cores = BZ

    bass2jax.install_neuronx_cc_hook()
    partition_name = nc.partition_id_tensor.name if nc.partition_id_tensor else None

    in_names, out_names, out_avals, zero_shapes = [], [], [], []
    for alloc in nc.m.functions[0].allocations:
        if not isinstance(alloc, mybir.MemoryLocationSet):
            continue
        name = alloc.memorylocations[0].name
        if alloc.kind == "ExternalInput":
            if name != partition_name:
                in_names.append(name)
        elif alloc.kind == "ExternalOutput":
            shape = tuple(alloc.tensor_shape)
            dtype = mybir.dt.np(alloc.dtype)
            out_names.append(name)
            out_avals.append(jax.core.ShapedArray(shape, dtype))
            zero_shapes.append((shape, dtype))
    n_params = len(in_names)
    all_names = list(in_names) + list(out_names)
    if partition_name is not None:
        all_names.append(partition_name)
    donate = tuple(range(n_params, n_params + len(out_names)))

    def _body(*args):
        operands = list(args)
        if partition_name is not None:
            operands.append(bass2jax.partition_id_tensor())
        outs = bass2jax._bass_exec_p.bind(
            *operands,
            out_avals=tuple(out_avals),
            in_names=tuple(all_names),
            out_names=tuple(out_names),
            lowering_input_output_aliases=(),
            sim_require_finite=True,
            sim_require_nnan=True,
            nc=nc,
        )
        return tuple(outs)

    devices = jax.devices()[:n_cores]
    mesh = Mesh(np.asarray(devices), ("core",))
    spec = PartitionSpec("core")
    sharded = jax.jit(
        shard_map(
            _body,
            mesh=mesh,
            in_specs=(spec,) * (n_params + len(out_names)),
            out_specs=(spec,) * len(out_names),
            check_rep=False,
        ),
        donate_argnums=donate,
        keep_unused=True,
    )
    shd = NamedSharding(mesh, spec)
    concat_in = [
        jax.device_put(
            np.concatenate([np.asarray(in_maps[c][nm]) for c in range(n_cores)], axis=0),
            shd,
        )
        for nm in in_names
    ]
    jax.block_until_ready(concat_in)

    def fresh_zeros():
        return [
            jax.device_put(np.zeros((n_cores * s[0], *s[1:]), d), shd)
            for (s, d) in zero_shapes
        ]

    times = []
    out = None
    for i in range(iters):
        z = fresh_zeros()
        jax.block_until_ready(z)
        t0 = time.perf_counter()
        out = sharded(*concat_in, *z)
        jax.block_until_ready(out)
        times.append(time.perf_counter() - t0)
    return min(times), times, np.asarray(out[0])
